# revision 1
# baseline (speedup 1.0000x reference)
"""Batched int8 GEMM with scaling for TRN2: out[b] = round(alpha * (a[b] @ b[b]^T)).

Shapes (hardcoded per the problem spec): a [64,1024,128] int8, b [64,1024,128] int8,
alpha fp32 scalar -> out [64,1024,1024] int32.

Strategy:
- Shard batch dim B=64 across 8 NeuronCores (8 batches/core), no communication.
- Host-side prep: transpose to a^T [B,K,M] / b^T [B,K,N] (K=128 on partitions, the
  layout the PE array needs for both operands) and cast int8 -> bf16, which is exact
  for [-128,127]. Products (<=2^14) and K=128-deep sums (<=2^21) are exact in the
  fp32 PSUM accumulator, so the GEMM is bit-exact.
- Per m-tile: two 128x128x512 matmuls into a 2-bank PSUM tile, then one fused
  epilogue op (mul-by-alpha + fp32->int cast; HW cast is round-to-nearest-even,
  matching jnp.round) alternating between VectorE and ScalarE.
- PE warmup matmuls before the first inputs land keep the HAM clock-gate at
  2.4 GHz; output DMAs rotate across the three descriptor queues.
- Device output is int16 when alpha bounds |out| < 32768 (always true for the spec's
  alpha=2^-7: |acc| <= 128*128*128 = 2^21 -> |out| <= 16384), halving the dominant
  HBM write traffic; host upcasts to int32.
"""

import sys

sys.path.insert(0, "/opt/trn_rl_repo")

from contextlib import ExitStack

import ml_dtypes
import numpy as np

import concourse.tile as tile
from concourse import bacc, mybir
from concourse.bass_utils import run_bass_kernel_spmd

B, M, N, K = 64, 1024, 1024, 128
N_CORES = 8
BPC = B // N_CORES  # batches per core
MT = 128  # m-tile (PSUM partition dim)
NT = 512  # n-tile (one PSUM bank of fp32)

ACC_MAX = 128 * 128 * K  # max |a@b^T| entry for int8 operands

_cache: dict = {}


def _build(alpha: float, out16: bool):
    out_dt = mybir.dt.int16 if out16 else mybir.dt.int32
    nc = bacc.Bacc(
        "TRN2", target_bir_lowering=False, debug=False, num_devices=N_CORES
    )
    # int8 inputs, upcast to bf16 during the SWDGE DMA (halves input HBM
    # traffic; HWDGE cannot cast, so all input loads ride the gpsimd queue).
    aT = nc.dram_tensor("aT", [BPC, K, M], mybir.dt.int8, kind="ExternalInput").ap()
    bT = nc.dram_tensor("bT", [BPC, K, N], mybir.dt.int8, kind="ExternalInput").ap()
    # bf16 copy of batch 0: loads via sync (earliest-firing engine, and HWDGE
    # cannot cast) so real matmuls start ~8us in and self-warm the HAM gate
    aT0 = nc.dram_tensor("aT0", [K, M], mybir.dt.bfloat16, kind="ExternalInput").ap()
    bT0 = nc.dram_tensor("bT0", [K, N], mybir.dt.bfloat16, kind="ExternalInput").ap()
    # tiled output layout [batch, partition, m-tile, n]: each partition's
    # m-rows are contiguous in DRAM, so output DMA runs are 8 KB instead of
    # 2 KB (4x fewer descriptors, longer HBM bursts); host un-tiles
    out_r = nc.dram_tensor(
        "out", [BPC, MT, M // MT, N], out_dt, kind="ExternalOutput"
    ).ap()

    with tile.TileContext(nc) as tc, ExitStack() as ctx:
        a_pool = ctx.enter_context(tc.tile_pool(name="a", bufs=1))
        b_pool = ctx.enter_context(tc.tile_pool(name="b", bufs=1))
        ps_pool = ctx.enter_context(tc.tile_pool(name="ps", bufs=4, space="PSUM"))
        o_pool = ctx.enter_context(tc.tile_pool(name="o", bufs=8))

        # PE warmup: dummy matmuls on a zeroed tile while the first inputs
        # load, so HAM un-throttles (1.2 -> 2.4 GHz) before real work and
        # batch 0 doesn't run at half clock. Reuses the psum pool slots.
        # (Measured: removing this and starting real MMs early-but-cold is
        # ~2.5us slower end-to-end.)
        warm_pool = ctx.enter_context(tc.tile_pool(name="warm", bufs=1))
        wz = warm_pool.tile([K, NT], mybir.dt.bfloat16)
        nc.vector.memset(wz[:], 0.0)
        for w in range(6):
            wps = ps_pool.tile([MT, N], mybir.dt.float32, tag="ps")
            for n in range(N // NT):
                nc.tensor.matmul(
                    wps[:, n * NT : (n + 1) * NT], wz[:, :MT], wz[:],
                    start=True, stop=True,
                )

        # all input loads issued up-front: batch 0 in bf16 chunks on sync
        # (earliest-firing engine); the rest as int8->bf16 cast-DMAs, ahead
        # of any output trigger in the gpsimd SWDGE FIFO so inputs never
        # starve
        ats, bts = [], []
        for i in range(BPC):
            at = a_pool.tile([K, M], mybir.dt.bfloat16, tag=f"a{i}")
            bt = b_pool.tile([K, N], mybir.dt.bfloat16, tag=f"b{i}")
            if i == 0:
                nc.sync.dma_start(at[:, :MT], aT0[:, :MT])
                nc.sync.dma_start(bt[:, :NT], bT0[:, :NT])
                nc.sync.dma_start(bt[:, NT:], bT0[:, NT:])
                nc.sync.dma_start(at[:, MT:], aT0[:, MT:])
            else:
                nc.gpsimd.dma_start(at[:], aT[i])  # int8 -> bf16 in DMA
                nc.gpsimd.dma_start(bt[:], bT[i])
            ats.append(at)
            bts.append(bt)

        MH = M // MT // 2  # m-tiles per output DMA (half batch = 1 MB)
        # output chunks rotate across all three DMA queues (SP ring, gpsimd
        # SWDGE, ACT ring) for drain parallelism
        out_engs = [nc.sync, nc.gpsimd, nc.scalar]
        chunk_idx = 0
        for i in range(BPC):
            at, bt = ats[i], bts[i]
            for h in range(2):
                ot = o_pool.tile([MT, MH, N], out_dt)
                for mh in range(MH):
                    m = h * MH + mh
                    ps = ps_pool.tile([MT, N], mybir.dt.float32)
                    for n in range(N // NT):
                        nc.tensor.matmul(
                            ps[:, n * NT : (n + 1) * NT],
                            at[:, m * MT : (m + 1) * MT],
                            bt[:, n * NT : (n + 1) * NT],
                            start=True,
                            stop=True,
                        )
                    # one fused epilogue op per m-tile, alternating engines
                    osl = ot[:, mh, :]
                    if m % 2 == 0:
                        nc.vector.tensor_scalar_mul(osl, ps[:], alpha)
                    else:
                        nc.scalar.mul(osl, ps[:], alpha)
                dst = out_r[i][:, h * MH : (h + 1) * MH]
                if 0 < i < BPC - 1:
                    eng = out_engs[chunk_idx % len(out_engs)]
                    chunk_idx += 1
                    eng.dma_start(dst, ot[:])
                elif i == 0 and h == 0:
                    # very first chunk is a single m-tile so the drain window
                    # opens as early as possible
                    for lo, hi in ((0, 1), (1, MH)):
                        eng = out_engs[chunk_idx % len(out_engs)]
                        chunk_idx += 1
                        eng.dma_start(dst[:, lo:hi], ot[:, lo:hi])
                else:
                    # first batch h=1 / last batch: quarter-DMAs to open the
                    # drain window early and cut the tail; the final quarters
                    # go to the rings that drain earliest (sync, then scalar)
                    MQ = MH // 2
                    for q in range(2):
                        if i == BPC - 1 and h == 1:
                            eng = nc.sync if q == 0 else nc.scalar
                        else:
                            eng = out_engs[chunk_idx % len(out_engs)]
                            chunk_idx += 1
                        eng.dma_start(
                            dst[:, q * MQ : (q + 1) * MQ],
                            ot[:, q * MQ : (q + 1) * MQ],
                        )

    nc.compile()
    return nc


def _get(alpha: float, out16: bool):
    key = (alpha, out16)
    if key not in _cache:
        _cache[key] = _build(alpha, out16)
    return _cache[key]


def make_in_maps(a: np.ndarray, b: np.ndarray):
    aT = np.ascontiguousarray(a.transpose(0, 2, 1))
    bT = np.ascontiguousarray(b.transpose(0, 2, 1))
    in_maps = []
    for c in range(N_CORES):
        asl = aT[c * BPC : (c + 1) * BPC]
        bsl = bT[c * BPC : (c + 1) * BPC]
        in_maps.append(
            {
                "aT": asl,
                "bT": bsl,
                "aT0": asl[0].astype(ml_dtypes.bfloat16),
                "bT0": bsl[0].astype(ml_dtypes.bfloat16),
            }
        )
    return in_maps


def kernel(a: np.ndarray, b: np.ndarray, alpha: np.ndarray) -> np.ndarray:
    alpha_f = float(np.asarray(alpha))
    out16 = abs(alpha_f) * ACC_MAX < 32767.5

    nc = _get(alpha_f, out16)
    in_maps = make_in_maps(a, b)
    res = run_bass_kernel_spmd(nc, in_maps, list(range(N_CORES))).results
    # un-tile [BPC, p, m, n] -> [BPC, (m p), n] and upcast
    out = np.concatenate([res[c]["out"] for c in range(N_CORES)], axis=0)
    out = out.transpose(0, 2, 1, 3).reshape(B, M, N)
    return out.astype(np.int32)



# revision 3
# speedup vs baseline: 1.0337x; 1.0337x over previous
"""Batched int8 GEMM with scaling for TRN2: out[b] = round(alpha * (a[b] @ b[b]^T)).

Shapes (hardcoded per the problem spec): a [64,1024,128] int8, b [64,1024,128] int8,
alpha fp32 scalar -> out [64,1024,1024] int32.

Strategy:
- Shard batch dim B=64 across 8 NeuronCores (8 batches/core), no communication.
- Host-side prep: transpose to a^T [B,K,M] / b^T [B,K,N] (K=128 on partitions, the
  layout the PE array needs for both operands). int8 -> bf16 cast happens inside the
  SWDGE DMA (exact for [-128,127]); K=128-deep dot products are exact in the fp32
  PSUM accumulator, so the GEMM is bit-exact.
- Fine-grained m-tile pipeline: per 128-row m-tile, two 128x128x512 matmuls into a
  2-bank PSUM tile, one fused epilogue op (mul-by-alpha + fp32->int cast, round-to-
  nearest-even = jnp.round) alternating VectorE/ScalarE, then a 256KB fully-contiguous
  output DMA alternating the sync/gpsimd queues (ScalarE stays dedicated to epilogue).
- Input cast-DMAs are dispatched just-in-time (two batches ahead) so the gpsimd queue
  is not clogged at t=0 and the first output chunk fires as early as possible.
- Device output is int16 when alpha bounds |out| < 32768 (true for alpha=2^-7:
  |acc| <= 2^21 -> |out| <= 16384), halving the dominant HBM write traffic; host
  upcasts to int32. Output DRAM layout [batch, m-tile, row, N] makes every chunk a
  contiguous 256KB block and host un-tiling a pure reshape.
"""

import sys

sys.path.insert(0, "/opt/trn_rl_repo")

from contextlib import ExitStack

import numpy as np

import concourse.tile as tile
from concourse import bacc, mybir
from concourse.bass_utils import run_bass_kernel_spmd

B, M, N, K = 64, 1024, 1024, 128
N_CORES = 8
BPC = B // N_CORES  # batches per core
MT = 128  # m-tile (PSUM partition dim)
NT = 512  # n-tile (one PSUM bank of fp32)
NMT = M // MT  # m-tiles per batch

ACC_MAX = 128 * 128 * K  # max |a@b^T| entry for int8 operands

_cache: dict = {}


def _build(alpha: float, out16: bool):
    out_dt = mybir.dt.int16 if out16 else mybir.dt.int32
    nc = bacc.Bacc(
        "TRN2", target_bir_lowering=False, debug=False, num_devices=N_CORES
    )
    # int8 inputs, upcast to bf16 during the SWDGE DMA (halves input HBM
    # traffic; HWDGE cannot cast, so all input loads ride the gpsimd queue).
    aT = nc.dram_tensor("aT", [BPC, K, M], mybir.dt.int8, kind="ExternalInput").ap()
    bT = nc.dram_tensor("bT", [BPC, K, N], mybir.dt.int8, kind="ExternalInput").ap()
    # m-tile-major output layout [batch, m-tile, row-in-tile, n]: each output
    # chunk is one fully contiguous 256KB block in DRAM and the host un-tile
    # is a plain reshape.
    out_r = nc.dram_tensor(
        "out", [BPC, NMT, MT, N], out_dt, kind="ExternalOutput"
    ).ap()

    with tile.TileContext(nc) as tc, ExitStack() as ctx:
        a_pool = ctx.enter_context(tc.tile_pool(name="a", bufs=1))
        b_pool = ctx.enter_context(tc.tile_pool(name="b", bufs=1))
        ps_pool = ctx.enter_context(tc.tile_pool(name="ps", bufs=4, space="PSUM"))
        o_pool = ctx.enter_context(tc.tile_pool(name="o", bufs=8))

        # All 8 batches stay resident in SBUF (4KB/partition total); tiles are
        # created up-front, loads dispatched just-in-time in the batch loop.
        ats = [
            a_pool.tile([K, M], mybir.dt.bfloat16, name=f"at{i}", tag=f"a{i}")
            for i in range(BPC)
        ]
        bts = [
            b_pool.tile([K, N], mybir.dt.bfloat16, name=f"bt{i}", tag=f"b{i}")
            for i in range(BPC)
        ]

        def load_batch(i):
            # b first: the first matmul of batch i needs all of b but only the
            # first m-tile slice of a.
            nc.gpsimd.dma_start(bts[i][:], bT[i])  # int8 -> bf16 in DMA
            nc.gpsimd.dma_start(ats[i][:], aT[i])

        load_batch(0)
        load_batch(1)

        tile_idx = 0
        out_engs = [nc.sync, nc.gpsimd]
        for i in range(BPC):
            if i + 2 < BPC:
                load_batch(i + 2)  # two batches of lead time
            at, bt = ats[i], bts[i]
            for m in range(NMT):
                ps = ps_pool.tile([MT, N], mybir.dt.float32)
                for n in range(N // NT):
                    nc.tensor.matmul(
                        ps[:, n * NT : (n + 1) * NT],
                        at[:, m * MT : (m + 1) * MT],
                        bt[:, n * NT : (n + 1) * NT],
                        start=True,
                        stop=True,
                    )
                ot = o_pool.tile([MT, N], out_dt)
                # fused scale + fp32->int cast (round-to-nearest-even), one op
                # per m-tile, alternating the two PSUM-capable engines
                if tile_idx % 2 == 0:
                    nc.vector.tensor_scalar_mul(ot[:], ps[:], alpha)
                else:
                    nc.scalar.mul(ot[:], ps[:], alpha)
                # 256KB contiguous chunk; alternate the two non-epilogue queues
                out_engs[tile_idx % 2].dma_start(out_r[i][m], ot[:])
                tile_idx += 1

    nc.compile()
    return nc


def _get(alpha: float, out16: bool):
    key = (alpha, out16)
    if key not in _cache:
        _cache[key] = _build(alpha, out16)
    return _cache[key]


def make_in_maps(a: np.ndarray, b: np.ndarray):
    aT = np.ascontiguousarray(a.transpose(0, 2, 1))
    bT = np.ascontiguousarray(b.transpose(0, 2, 1))
    in_maps = []
    for c in range(N_CORES):
        in_maps.append(
            {
                "aT": aT[c * BPC : (c + 1) * BPC],
                "bT": bT[c * BPC : (c + 1) * BPC],
            }
        )
    return in_maps


def kernel(a: np.ndarray, b: np.ndarray, alpha: np.ndarray) -> np.ndarray:
    alpha_f = float(np.asarray(alpha))
    out16 = abs(alpha_f) * ACC_MAX < 32767.5

    nc = _get(alpha_f, out16)
    in_maps = make_in_maps(a, b)
    res = run_bass_kernel_spmd(nc, in_maps, list(range(N_CORES))).results
    # [BPC, NMT, MT, N] -> [BPC, M, N]: rows are already in order, pure reshape
    out = np.concatenate([res[c]["out"] for c in range(N_CORES)], axis=0)
    out = out.reshape(B, M, N)
    return out.astype(np.int32)
